# revision 58
# baseline (speedup 1.0000x reference)
"""Block-sparse (local-window) attention on 8 Trainium2 NeuronCores.

Baseline-derived kernel with layered optimizations:
  L1 (always): 2-head packing -- the two heads of a head-pair live on
      the two 64-partition halves of qT/kT, so nothing is duplicated
      host-side and input DMA halves.
  L2 (BS_L2): PSUM->SBUF evictions alternate ScalarE/DVE.
  L3 (BS_L3): exp alternates ScalarE true Exp and DVE Schraudolph
      fast-exp (int16 bit pattern == bf16(exp(s/8))).
  L4 (BS_L4): output shipped as bf16 packed in f32 words; host
      normalizes (divide by denominator row) and converts.
  L5 (BS_L5): small first-wave DMAs so compute starts earlier.

Scores are computed TRANSPOSED per 128-key chunk over its 384-query
window (st[kc, q]); exp batched per quad; out-of-window 64x64 corners
zeroed on GPSIMD after exp; AV uses vp=[V | ones] stationary so row 64
of the output is the softmax denominator.
"""

import os
import numpy as np
import ml_dtypes

import concourse.bass as bass
import concourse.mybir as mybir
import concourse.tile as tile
import concourse.bass_utils as _bu
from concourse.bass_utils import run_bass_kernel_spmd

B, S, H, D = 2, 4096, 16, 64
N_CORES = 8
GH = B * H
G = GH // N_CORES          # 4 pairs per core
HP = G // 2                # 2 head-pairs per core
NT = S // 128              # 32 query tiles / key chunks of 128
QUADS = NT // 4            # 8 quads of 4 query tiles
_PERM = (0, 2, 1, 3)
BF16 = mybir.dt.bfloat16
I16 = mybir.dt.int16
F32 = mybir.dt.float32

def _flag(name, default):
    v = os.environ.get(name)
    if v is None:
        return default
    return v not in ("", "0")


L2 = _flag("BS_L2", True)
L3 = _flag("BS_L3", True)
L4 = _flag("BS_L4", True)
L5 = _flag("BS_L5", True)
L6 = _flag("BS_L6", False)  # interleaved-heads ST units (crashes on HW)
L7 = _flag("BS_L7", False)  # interleave the two pairs of a head-pair
AVLAG = int(os.environ.get("BS_AVLAG", "3"))  # ST->AV emission lag (units)

# Schraudolph fast-exp: i16 = round(FE_A * s + FE_B); bits = bf16(exp(s/8))
FE_A = (128.0 / np.log(2.0)) / 8.0
FE_B = 16244.5

# exp split among the 32 (pair, quad) units; ScalarE is 1.25x faster.
# Counts tunable via env for balance experiments.
_NV = int(os.environ.get("BS_NV", "12"))   # units on DVE fast-exp
_NSE = int(os.environ.get("BS_NSE", "12"))  # evicts on ScalarE
EXP_ENGINE = [
    'V' if (i * _NV) // 32 != ((i + 1) * _NV) // 32 else 'S' for i in range(32)
]
EVICT_ENGINE = [
    'S' if (i * _NSE) // 32 != ((i + 1) * _NSE) // 32 else 'V' for i in range(32)
]

_nc_cache = None

_NO_SPLIT_TYPES = (
    "InstEventSemaphore",
    "InstCall",
    "InstUnconditionalBranch",
    "InstConditionalBranch",
    "InstISA",
    "InstRegisterMove",
    "InstNoOp",
    "InstTriggerDma",
)


def _split_excess_waits(nc, budget=1):
    f = nc.m.functions[0]
    for bb in f.blocks:
        insts = list(bb.instructions)
        out = []
        changed = False
        for ins in insts:
            si = ins.sync_info
            if (
                type(ins).__name__ not in _NO_SPLIT_TYPES
                and si is not None
                and len(si.on_wait) > budget
            ):
                waits = list(si.on_wait)
                extra, keep = waits[:-budget], waits[-budget:]
                for w in extra:
                    nop = mybir.InstNoOp(
                        name=nc.get_next_instruction_name(),
                        sync_info=mybir.SyncInfo(on_wait=[w], on_update=[]),
                        bass_nofuse=True,
                        engine=ins.engine,
                    )
                    out.append(nop)
                    changed = True
                ins.sync_info = mybir.SyncInfo(
                    on_wait=keep, on_update=list(si.on_update)
                )
            out.append(ins)
        if changed:
            bb.instructions = out
    return nc


_PRUNABLE_UPDATERS = (
    "InstMatmult",
    "InstActivation",
    "InstReciprocal",
    "InstTensorScalarPtr",
    "InstTensorScalar",
    "InstMemset",
)


def _prune_sem_updates(nc):
    f = nc.m.functions[0]
    all_insts = [ins for bb in f.blocks for ins in bb.instructions]
    referenced = {}
    for ins in all_insts:
        si = ins.sync_info
        if si:
            for w in si.on_wait:
                referenced.setdefault(w.id, set()).add(w.wait_value)
    from collections import defaultdict

    upd = defaultdict(list)
    untouchable = set()
    for ins in all_insts:
        si = ins.sync_info
        if not si:
            continue
        for u in si.on_update:
            upd[u.id].append(ins)
            if type(ins).__name__ not in _PRUNABLE_UPDATERS or u.update_value != 1:
                untouchable.add(u.id)
    for sem_id, lst in upd.items():
        if sem_id in untouchable:
            continue
        n = len(lst)
        refs = referenced.get(sem_id, set())
        kept = sorted(v for v in refs if 1 <= v <= n)
        if not kept or kept[-1] != n:
            kept.append(n)
        kept_set = set(kept)
        rank = {v: i + 1 for i, v in enumerate(kept)}
        for tick, ins in enumerate(lst, start=1):
            if tick in kept_set:
                continue
            si = ins.sync_info
            ins.sync_info = mybir.SyncInfo(
                on_wait=list(si.on_wait),
                on_update=[u for u in si.on_update if u.id != sem_id],
            )
        for ins in all_insts:
            si = ins.sync_info
            if not si or not any(w.id == sem_id for w in si.on_wait):
                continue
            new_waits = []
            for w in si.on_wait:
                if w.id == sem_id:
                    w = mybir.SyncWait(
                        sync_type=w.sync_type,
                        id=w.id,
                        ant_name=w.ant_name,
                        wait_mode=w.wait_mode,
                        wait_value=rank[w.wait_value],
                        wait_reg=w.wait_reg,
                    )
                new_waits.append(w)
            ins.sync_info = mybir.SyncInfo(
                on_wait=new_waits, on_update=list(si.on_update)
            )
    return nc


def _build_bass():
    nc = bass.Bass()
    qT_d = nc.declare_dram_parameter("qT", [HP, 128, S], BF16, isOutput=False)
    kT_d = nc.declare_dram_parameter("kT", [HP, 128, S], BF16, isOutput=False)
    vp_d = nc.declare_dram_parameter("vp", [G, 128, NT, D + 1], BF16, isOutput=False)
    if L4:
        out_d = nc.declare_dram_parameter(
            "out", [G, D + 1, S // 2], F32, isOutput=True
        )
    else:
        out_d = nc.declare_dram_parameter("out", [G, D + 1, S], F32, isOutput=True)

    with tile.TileContext(nc) as tc:
        with (
            tc.tile_pool(name="const", bufs=1) as c_pool,
            tc.tile_pool(name="qk", bufs=2) as qk_pool,
            tc.tile_pool(name="vpool", bufs=4 if (L6 or L7) else 2) as v_pool,
            tc.tile_pool(name="opool", bufs=4 if (L6 or L7) else 2) as o_pool,
            tc.tile_pool(
                name="ppool", bufs=max(8 if (L6 or L7) else 4, AVLAG + 3)
            ) as p_pool,
            tc.tile_pool(name="stps", bufs=2, space="PSUM") as st_pool,
            tc.tile_pool(name="otps", bufs=2, space="PSUM") as ot_pool,
        ):
            bias0 = c_pool.tile([128, 1], F32, name="bias0")
            nc.vector.memset(bias0, 0.0)
            scratch0 = c_pool.tile([128, 1], F32, name="scratch0")
            nc.scalar.activation(
                scratch0, bias0, mybir.ActivationFunctionType.Exp, bias=bias0
            )

            if L7:
                # Interleave the two pairs of each head-pair quad-by-quad:
                # consecutive ST matmuls alternate PE row halves, so the
                # next strip's LDWEIGHTS overlaps the in-flight matmul.
                units = [
                    (2 * hp + h, q)
                    for hp in range(HP)
                    for q in range(QUADS)
                    for h in range(2)
                ]
            else:
                units = [(g, q) for g in range(G) for q in range(QUADS)]
            uidx = {gq: j for j, gq in enumerate(units)}
            qkv = {}
            hp_qk = {}
            p_t = {}

            def load_hp(hp):
                qT_sb = qk_pool.tile([128, S], BF16, tag="qT", name=f"qT{hp}")
                kT_sb = qk_pool.tile([128, S], BF16, tag="kT", name=f"kT{hp}")
                C0 = 1024
                nc.sync.dma_start(out=qT_sb[:, 0:C0], in_=qT_d[hp][:, 0:C0])
                nc.sync.dma_start(out=kT_sb[:, 0:C0], in_=kT_d[hp][:, 0:C0])
                nc.sync.dma_start(out=qT_sb[:, C0 : S // 2], in_=qT_d[hp][:, C0 : S // 2])
                nc.sync.dma_start(out=kT_sb[:, C0 : S // 2], in_=kT_d[hp][:, C0 : S // 2])
                nc.sync.dma_start(out=qT_sb[:, S // 2 :], in_=qT_d[hp][:, S // 2 :])
                nc.sync.dma_start(out=kT_sb[:, S // 2 :], in_=kT_d[hp][:, S // 2 :])
                hp_qk[hp] = (qT_sb, kT_sb)

            def load_pair(g):
                vp_sb = v_pool.tile([128, NT, D + 1], BF16, tag="vp", name=f"vp{g}")
                nc.sync.dma_start(
                    out=vp_sb[:, 0 : NT // 2, :], in_=vp_d[g][:, 0 : NT // 2, :]
                )
                nc.sync.dma_start(
                    out=vp_sb[:, NT // 2 :, :], in_=vp_d[g][:, NT // 2 :, :]
                )
                out_sb = o_pool.tile(
                    [D + 1, S], BF16 if L4 else F32, tag="osb", name=f"o{g}"
                )
                qkv[g] = (vp_sb, out_sb)

            def emit_st7(j):
                g, quad = units[j]
                hp = g // 2
                rh = 64 * (g % 2)
                if j == 0:
                    load_hp(0)
                    load_pair(0)
                    load_pair(1)
                if j == 8:
                    # prefetch the second head-pair mid-way through the
                    # first so its 3MB doesn't stall the handover
                    load_hp(1)
                    load_pair(2)
                    load_pair(3)
                qT_sb, kT_sb = hp_qk[hp]
                vp_sb, out_sb = qkv[g]
                st = st_pool.tile([128, 1536], F32, tag="st", name=f"st{j}")
                p_sb = p_pool.tile([128, 1536], BF16, tag="p", name=f"p{j}")
                p_t[j] = p_sb
                for s in range(4):
                    c = quad * 4 + s
                    base = _PERM[s] * 384
                    t_lo = max(0, c - 1)
                    t_hi = min(NT, c + 2)
                    p0 = base + (t_lo - (c - 1)) * 128
                    bnd = base + (t_hi - (c - 1)) * 128
                    while p0 < bnd:
                        p1 = min(bnd, (p0 // 512 + 1) * 512)
                        q0 = (c - 1) * 128 + (p0 - base)
                        nc.tensor.matmul(
                            st[:, p0:p1],
                            lhsT=kT_sb[rh : rh + 64, c * 128 : (c + 1) * 128],
                            rhs=qT_sb[rh : rh + 64, q0 : q0 + (p1 - p0)],
                            start=True,
                            stop=True,
                        )
                        p0 = p1
                lo = 128 if quad == 0 else 0
                hi = 1536 - 128 if quad == QUADS - 1 else 1536
                if L3 and EXP_ENGINE[j] == 'V':
                    nc.vector.tensor_scalar(
                        p_sb[:, lo:hi].bitcast(I16),
                        st[:, lo:hi],
                        FE_A,
                        FE_B,
                        mybir.AluOpType.mult,
                        mybir.AluOpType.add,
                    )
                else:
                    nc.scalar.activation(
                        p_sb[:, lo:hi],
                        st[:, lo:hi],
                        mybir.ActivationFunctionType.Exp,
                        bias=bias0,
                        scale=1.0 / np.sqrt(D).item(),
                    )
                # strip 0's low corner gates the PREVIOUS quad's AV tail
                # (cross-quad chunk read); emit the low corners first.
                for s in range(4):
                    c = quad * 4 + s
                    base = _PERM[s] * 384
                    if c >= 1:
                        nc.gpsimd.memset(p_sb[64:128, base : base + 64], 0.0)
                for s in range(4):
                    c = quad * 4 + s
                    base = _PERM[s] * 384
                    if c <= NT - 2:
                        nc.gpsimd.memset(p_sb[0:64, base + 320 : base + 384], 0.0)

            def emit_st(j):
                g, quad = units[j]
                hp = g // 2
                rh = 64 * (g % 2)
                if quad == 0 and g % 2 == 0:
                    qT_sb = qk_pool.tile([128, S], BF16, tag="qT", name=f"qT{hp}")
                    kT_sb = qk_pool.tile([128, S], BF16, tag="kT", name=f"kT{hp}")
                    C0 = 1024
                    if hp == 0:
                        # the very first ST waits on these columns; split
                        # them across queues (a single dma_start rides one
                        # ~17GB/s queue) so compute starts ~4us earlier
                        for a, b in ((0, 128), (128, 256), (256, 384), (384, 512)):
                            nc.sync.dma_start(out=qT_sb[:, a:b], in_=qT_d[hp][:, a:b])
                            nc.sync.dma_start(out=kT_sb[:, a:b], in_=kT_d[hp][:, a:b])
                        nc.sync.dma_start(out=qT_sb[:, 512:C0], in_=qT_d[hp][:, 512:C0])
                        nc.sync.dma_start(out=kT_sb[:, 512:C0], in_=kT_d[hp][:, 512:C0])
                    else:
                        nc.sync.dma_start(out=qT_sb[:, 0:C0], in_=qT_d[hp][:, 0:C0])
                        nc.sync.dma_start(out=kT_sb[:, 0:C0], in_=kT_d[hp][:, 0:C0])
                    nc.sync.dma_start(
                        out=qT_sb[:, C0 : S // 2], in_=qT_d[hp][:, C0 : S // 2]
                    )
                    nc.sync.dma_start(
                        out=kT_sb[:, C0 : S // 2], in_=kT_d[hp][:, C0 : S // 2]
                    )
                    nc.sync.dma_start(out=qT_sb[:, S // 2 :], in_=qT_d[hp][:, S // 2 :])
                    nc.sync.dma_start(out=kT_sb[:, S // 2 :], in_=kT_d[hp][:, S // 2 :])
                    hp_qk[hp] = (qT_sb, kT_sb)
                if quad == 0:
                    vp_sb = v_pool.tile([128, NT, D + 1], BF16, tag="vp", name=f"vp{g}")
                    nc.sync.dma_start(
                        out=vp_sb[:, 0 : NT // 2, :], in_=vp_d[g][:, 0 : NT // 2, :]
                    )
                    nc.sync.dma_start(
                        out=vp_sb[:, NT // 2 :, :], in_=vp_d[g][:, NT // 2 :, :]
                    )
                    out_sb = o_pool.tile(
                        [D + 1, S], BF16 if L4 else F32, tag="osb", name=f"o{g}"
                    )
                    qkv[g] = (vp_sb, out_sb)
                qT_sb, kT_sb = hp_qk[hp]
                vp_sb, out_sb = qkv[g]
                st = st_pool.tile([128, 1536], F32, tag="st", name=f"st{j}")
                p_sb = p_pool.tile([128, 1536], BF16, tag="p", name=f"p{j}")
                p_t[j] = p_sb
                for s in range(4):
                    c = quad * 4 + s
                    base = _PERM[s] * 384
                    t_lo = max(0, c - 1)
                    t_hi = min(NT, c + 2)
                    p0 = base + (t_lo - (c - 1)) * 128
                    bnd = base + (t_hi - (c - 1)) * 128
                    pieces = []
                    while p0 < bnd:
                        p1 = min(bnd, (p0 // 512 + 1) * 512)
                        pieces.append((p0, p1))
                        p0 = p1
                    for p0, p1 in pieces:
                        q0 = (c - 1) * 128 + (p0 - base)
                        nc.tensor.matmul(
                            st[:, p0:p1],
                            lhsT=kT_sb[rh : rh + 64, c * 128 : (c + 1) * 128],
                            rhs=qT_sb[rh : rh + 64, q0 : q0 + (p1 - p0)],
                            start=True,
                            stop=True,
                        )
                lo = 128 if quad == 0 else 0
                hi = 1536 - 128 if quad == QUADS - 1 else 1536
                if L3 and EXP_ENGINE[j] == 'V':
                    nc.vector.tensor_scalar(
                        p_sb[:, lo:hi].bitcast(I16),
                        st[:, lo:hi],
                        FE_A,
                        FE_B,
                        mybir.AluOpType.mult,
                        mybir.AluOpType.add,
                    )
                else:
                    nc.scalar.activation(
                        p_sb[:, lo:hi],
                        st[:, lo:hi],
                        mybir.ActivationFunctionType.Exp,
                        bias=bias0,
                        scale=1.0 / np.sqrt(D).item(),
                    )
                for s in range(4):
                    c = quad * 4 + s
                    base = _PERM[s] * 384
                    if c >= 1:
                        nc.gpsimd.memset(p_sb[64:128, base : base + 64], 0.0)
                for s in range(4):
                    c = quad * 4 + s
                    base = _PERM[s] * 384
                    if c <= NT - 2:
                        nc.gpsimd.memset(p_sb[0:64, base + 320 : base + 384], 0.0)

            def emit_av(j):
                g, quad = units[j]
                vp_sb, out_sb = qkv[g]
                ot = ot_pool.tile([D + 1, 512], F32, tag="ot", name=f"ot{j}")
                t0 = quad * 4
                mms = []
                for c in range(max(0, t0 - 1), min(NT, t0 + 5)):
                    t_lo = max(t0, c - 1, 0)
                    t_hi = min(t0 + 4, c + 2, NT)
                    if t_lo >= t_hi:
                        continue
                    pq = p_t[uidx[(g, c // 4)]]
                    r0 = _PERM[c % 4] * 384 + (t_lo - (c - 1)) * 128
                    r1 = _PERM[c % 4] * 384 + (t_hi - (c - 1)) * 128
                    mms.append(
                        (
                            ot[:, (t_lo - t0) * 128 : (t_hi - t0) * 128],
                            vp_sb[:, c, :],
                            pq[:, r0:r1],
                        )
                    )
                for i, (o, w, r) in enumerate(mms):
                    nc.tensor.matmul(
                        o,
                        lhsT=w,
                        rhs=r,
                        start=(i == 0),
                        stop=(i == len(mms) - 1),
                        skip_group_check=True,
                    )
                if L2 and EVICT_ENGINE[j] == 'S':
                    nc.scalar.copy(out_sb[:, quad * 512 : (quad + 1) * 512], ot[:, :])
                else:
                    nc.vector.tensor_copy(
                        out_sb[:, quad * 512 : (quad + 1) * 512], ot[:, :]
                    )
                p_t.pop(j - (4 if L7 else 1), None)
                if quad % 2 == 1:
                    # the final 2-quad chunk is the kernel's drain tail;
                    # split it so the last transfer parallelizes
                    if quad == QUADS - 1:
                        spans = [
                            ((quad - 1) * 512 + 256 * t, (quad - 1) * 512 + 256 * (t + 1))
                            for t in range(4)
                        ]
                    else:
                        spans = [((quad - 1) * 512, (quad + 1) * 512)]
                    for a, b in spans:
                        if L4:
                            nc.sync.dma_start(
                                out=out_d[g][:, a // 2 : b // 2],
                                in_=out_sb[:, a:b].bitcast(F32),
                            )
                        else:
                            nc.sync.dma_start(
                                out=out_d[g][:, a:b], in_=out_sb[:, a:b]
                            )

            # L6: units of (head-pair, 2 chunks) with the two heads' strips
            # interleaved so consecutive ST matmuls alternate PE row halves
            # (next strip's LDWEIGHTS overlaps the in-flight matmul).
            # st/p layout: col = h*768 + s*384 + w, chunk c = 2u+s.
            def emit_unit6(g_idx):
                hp, u = g_idx % 2, g_idx // 2
                if u == 0:
                    qT_sb = qk_pool.tile([128, S], BF16, tag="qT", name=f"qT{hp}")
                    kT_sb = qk_pool.tile([128, S], BF16, tag="kT", name=f"kT{hp}")
                    C0 = 1024
                    nc.sync.dma_start(out=qT_sb[:, 0:C0], in_=qT_d[hp][:, 0:C0])
                    nc.sync.dma_start(out=kT_sb[:, 0:C0], in_=kT_d[hp][:, 0:C0])
                    nc.sync.dma_start(out=qT_sb[:, C0 : S // 2], in_=qT_d[hp][:, C0 : S // 2])
                    nc.sync.dma_start(out=kT_sb[:, C0 : S // 2], in_=kT_d[hp][:, C0 : S // 2])
                    nc.sync.dma_start(out=qT_sb[:, S // 2 :], in_=qT_d[hp][:, S // 2 :])
                    nc.sync.dma_start(out=kT_sb[:, S // 2 :], in_=kT_d[hp][:, S // 2 :])
                    hp_qk[hp] = (qT_sb, kT_sb)
                    for h in range(2):
                        g = 2 * hp + h
                        vp_sb = v_pool.tile(
                            [128, NT, D + 1], BF16, tag="vp", name=f"vp{g}"
                        )
                        nc.sync.dma_start(
                            out=vp_sb[:, 0 : NT // 2, :], in_=vp_d[g][:, 0 : NT // 2, :]
                        )
                        nc.sync.dma_start(
                            out=vp_sb[:, NT // 2 :, :], in_=vp_d[g][:, NT // 2 :, :]
                        )
                        out_sb = o_pool.tile(
                            [D + 1, S], BF16 if L4 else F32, tag="osb", name=f"o{g}"
                        )
                        qkv[g] = (vp_sb, out_sb)
                qT_sb, kT_sb = hp_qk[hp]
                st = st_pool.tile([128, 1536], F32, tag="st", name=f"st{g_idx}")
                p_sb = p_pool.tile([128, 1536], BF16, tag="p", name=f"p{g_idx}")
                p_t[g_idx] = p_sb
                for s in range(2):
                    c = 2 * u + s
                    for h in range(2):
                        base = h * 768 + s * 384
                        t_lo = max(0, c - 1)
                        t_hi = min(NT, c + 2)
                        p0 = base + (t_lo - (c - 1)) * 128
                        bnd = base + (t_hi - (c - 1)) * 128
                        while p0 < bnd:
                            p1 = min(bnd, (p0 // 512 + 1) * 512)
                            q0 = (c - 1) * 128 + (p0 - base)
                            nc.tensor.matmul(
                                st[:, p0:p1],
                                lhsT=kT_sb[64 * h : 64 * h + 64, c * 128 : (c + 1) * 128],
                                rhs=qT_sb[64 * h : 64 * h + 64, q0 : q0 + (p1 - p0)],
                                start=True,
                                stop=True,
                            )
                            p0 = p1
                # exp; first/last units trim the never-written hole columns
                # (uninitialized PSUM reads can fault the device).
                if u == 0:
                    ranges = [(128, 768), (896, 1536)]
                elif u == NT // 2 - 1:
                    ranges = [(0, 640), (768, 1408)]
                else:
                    ranges = [(0, 1536)]
                for lo, hi in ranges:
                    if L3 and EXP_ENGINE[g_idx] == 'V':
                        nc.vector.tensor_scalar(
                            p_sb[:, lo:hi].bitcast(I16),
                            st[:, lo:hi],
                            FE_A,
                            FE_B,
                            mybir.AluOpType.mult,
                            mybir.AluOpType.add,
                        )
                    else:
                        nc.scalar.activation(
                            p_sb[:, lo:hi],
                            st[:, lo:hi],
                            mybir.ActivationFunctionType.Exp,
                            bias=bias0,
                            scale=1.0 / np.sqrt(D).item(),
                        )
                for s in range(2):
                    c = 2 * u + s
                    for h in range(2):
                        base = h * 768 + s * 384
                        if c <= NT - 2:
                            nc.gpsimd.memset(p_sb[0:64, base + 320 : base + 384], 0.0)
                        if c >= 1:
                            nc.gpsimd.memset(p_sb[64:128, base : base + 64], 0.0)

            av6_count = [0]

            def emit_av6(hp, h, jq):
                g = 2 * hp + h
                vp_sb, out_sb = qkv[g]
                ot = ot_pool.tile([D + 1, 512], F32, tag="ot", name=f"ot{g}_{jq}")
                t0 = 4 * jq
                mms = []
                for c in range(max(0, t0 - 1), min(NT, t0 + 5)):
                    t_lo = max(t0, c - 1, 0)
                    t_hi = min(t0 + 4, c + 2, NT)
                    if t_lo >= t_hi:
                        continue
                    pq = p_t[2 * (c // 2) + hp]
                    base = h * 768 + (c % 2) * 384
                    r0 = base + (t_lo - (c - 1)) * 128
                    r1 = base + (t_hi - (c - 1)) * 128
                    mms.append(
                        (
                            ot[:, (t_lo - t0) * 128 : (t_hi - t0) * 128],
                            vp_sb[:, c, :],
                            pq[:, r0:r1],
                        )
                    )
                for i, (o, w, r) in enumerate(mms):
                    nc.tensor.matmul(
                        o,
                        lhsT=w,
                        rhs=r,
                        start=(i == 0),
                        stop=(i == len(mms) - 1),
                        skip_group_check=True,
                    )
                ev = av6_count[0]
                av6_count[0] += 1
                if L2 and EVICT_ENGINE[ev % 32] == 'S':
                    nc.scalar.copy(out_sb[:, jq * 512 : (jq + 1) * 512], ot[:, :])
                else:
                    nc.vector.tensor_copy(
                        out_sb[:, jq * 512 : (jq + 1) * 512], ot[:, :]
                    )
                if jq % 2 == 1:
                    sl = slice((jq - 1) * 512, (jq + 1) * 512)
                    if L4:
                        f_sl = slice((jq - 1) * 256, (jq + 1) * 256)
                        nc.sync.dma_start(
                            out=out_d[g][:, f_sl], in_=out_sb[:, sl].bitcast(F32)
                        )
                    else:
                        nc.sync.dma_start(out=out_d[g][:, sl], in_=out_sb[:, sl])

            if L6:
                av_fifo = []
                for g_idx in range(2 * (NT // 2)):
                    emit_unit6(g_idx)
                    hp, u = g_idx % 2, g_idx // 2
                    if u >= 2 and u % 2 == 0:
                        jq = u // 2 - 1
                        av_fifo.append((hp, 0, jq))
                        av_fifo.append((hp, 1, jq))
                    if av_fifo:
                        emit_av6(*av_fifo.pop(0))
                    p_t.pop(g_idx - 8, None)
                av_fifo.append((0, 0, QUADS - 1))
                av_fifo.append((0, 1, QUADS - 1))
                av_fifo.append((1, 0, QUADS - 1))
                av_fifo.append((1, 1, QUADS - 1))
                for av in av_fifo:
                    emit_av6(*av)
            else:
                st_fn = emit_st7 if L7 else emit_st
                for j in range(len(units)):
                    st_fn(j)
                    if j >= AVLAG:
                        emit_av(j - AVLAG)
                for jj in range(len(units) - AVLAG, len(units)):
                    emit_av(jj)
    _split_excess_waits(nc)
    return _prune_sem_updates(nc)


def _prep_inputs(q, k, v):
    bf16 = ml_dtypes.bfloat16
    qb = np.ascontiguousarray(np.asarray(q).transpose(0, 2, 1, 3).reshape(GH, S, D))
    kb = np.ascontiguousarray(np.asarray(k).transpose(0, 2, 1, 3).reshape(GH, S, D))
    vb = np.ascontiguousarray(np.asarray(v).transpose(0, 2, 1, 3).reshape(GH, S, D))

    qT = np.ascontiguousarray(qb.transpose(0, 2, 1)).astype(bf16).reshape(GH // 2, 128, S)
    kT = np.ascontiguousarray(kb.transpose(0, 2, 1)).astype(bf16).reshape(GH // 2, 128, S)
    v4 = vb.reshape(GH, NT, 128, D).transpose(0, 2, 1, 3)
    vp = np.empty((GH, 128, NT, D + 1), dtype=bf16)
    vp[..., :D] = v4.astype(bf16)
    vp[..., D] = np.array(1.0, dtype=bf16)

    in_maps = []
    for cc in range(N_CORES):
        in_maps.append(
            {
                "qT": np.ascontiguousarray(qT[cc * HP : (cc + 1) * HP]),
                "kT": np.ascontiguousarray(kT[cc * HP : (cc + 1) * HP]),
                "vp": np.ascontiguousarray(vp[cc * G : (cc + 1) * G]),
            }
        )
    return in_maps


def _assemble_output(results):
    o = np.concatenate([np.asarray(r["out"]) for r in results], axis=0)
    if L4:
        o = o.view(ml_dtypes.bfloat16).astype(np.float32)  # [GH, D+1, S]
    o = o[:, :D, :] / o[:, D : D + 1, :]
    o = o.transpose(0, 2, 1)
    o = o.reshape(B, H, S, D).transpose(0, 2, 1, 3)
    return np.ascontiguousarray(o.astype(np.float32))


def _run(q, k, v, trace=False, tmpdir=None):
    global _nc_cache
    if _nc_cache is None:
        _nc_cache = _build_bass()
    in_maps = _prep_inputs(q, k, v)
    res = run_bass_kernel_spmd(
        _nc_cache, in_maps, core_ids=list(range(N_CORES)), trace=trace, tmpdir=tmpdir
    )
    return _assemble_output(res.results), res.exec_time_ns


def kernel(q, k, v):
    out, _ = _run(q, k, v)
    return out


# revision 59
# speedup vs baseline: 1.0843x; 1.0843x over previous
"""Block-sparse (local-window) attention on 8 Trainium2 NeuronCores.

Baseline-derived kernel with layered optimizations:
  L1 (always): 2-head packing -- the two heads of a head-pair live on
      the two 64-partition halves of qT/kT, so nothing is duplicated
      host-side and input DMA halves.
  L2 (BS_L2): PSUM->SBUF evictions alternate ScalarE/DVE.
  L3 (BS_L3): exp alternates ScalarE true Exp and DVE Schraudolph
      fast-exp (int16 bit pattern == bf16(exp(s/8))).
  L4 (BS_L4): output shipped as bf16 packed in f32 words; host
      normalizes (divide by denominator row) and converts.
  L5 (BS_L5): small first-wave DMAs so compute starts earlier.

Scores are computed TRANSPOSED per 128-key chunk over its 384-query
window (st[kc, q]); exp batched per quad; out-of-window 64x64 corners
zeroed on GPSIMD after exp; AV uses vp=[V | ones] stationary so row 64
of the output is the softmax denominator.
"""

import os
import numpy as np
import ml_dtypes

import concourse.bass as bass
import concourse.mybir as mybir
import concourse.tile as tile
import concourse.bass_utils as _bu
from concourse.bass_utils import run_bass_kernel_spmd

B, S, H, D = 2, 4096, 16, 64
N_CORES = 8
GH = B * H
G = GH // N_CORES          # 4 pairs per core
HP = G // 2                # 2 head-pairs per core
NT = S // 128              # 32 query tiles / key chunks of 128
QUADS = NT // 4            # 8 quads of 4 query tiles
_PERM = (0, 2, 1, 3)
BF16 = mybir.dt.bfloat16
I16 = mybir.dt.int16
F32 = mybir.dt.float32

def _flag(name, default):
    v = os.environ.get(name)
    if v is None:
        return default
    return v not in ("", "0")


L2 = _flag("BS_L2", True)
L3 = _flag("BS_L3", True)
L4 = _flag("BS_L4", True)
L5 = _flag("BS_L5", True)
L6 = _flag("BS_L6", False)  # interleaved-heads ST units (crashes on HW)
L7 = _flag("BS_L7", False)  # interleave the two pairs of a head-pair
AVLAG = int(os.environ.get("BS_AVLAG", "3"))  # ST->AV emission lag (units)

# Schraudolph fast-exp: i16 = round(FE_A * s + FE_B); bits = bf16(exp(s/8))
FE_A = (128.0 / np.log(2.0)) / 8.0
FE_B = 16244.5

# exp split among the 32 (pair, quad) units; ScalarE is 1.25x faster.
# Counts tunable via env for balance experiments.
_NV = int(os.environ.get("BS_NV", "12"))   # units on DVE fast-exp
_NSE = int(os.environ.get("BS_NSE", "12"))  # evicts on ScalarE
EXP_ENGINE = [
    'V' if (i * _NV) // 32 != ((i + 1) * _NV) // 32 else 'S' for i in range(32)
]
EVICT_ENGINE = [
    'S' if (i * _NSE) // 32 != ((i + 1) * _NSE) // 32 else 'V' for i in range(32)
]

_nc_cache = None

_NO_SPLIT_TYPES = (
    "InstEventSemaphore",
    "InstCall",
    "InstUnconditionalBranch",
    "InstConditionalBranch",
    "InstISA",
    "InstRegisterMove",
    "InstNoOp",
    "InstTriggerDma",
)


def _split_excess_waits(nc, budget=1):
    f = nc.m.functions[0]
    for bb in f.blocks:
        insts = list(bb.instructions)
        out = []
        changed = False
        for ins in insts:
            si = ins.sync_info
            if (
                type(ins).__name__ not in _NO_SPLIT_TYPES
                and si is not None
                and len(si.on_wait) > budget
            ):
                waits = list(si.on_wait)
                extra, keep = waits[:-budget], waits[-budget:]
                for w in extra:
                    nop = mybir.InstNoOp(
                        name=nc.get_next_instruction_name(),
                        sync_info=mybir.SyncInfo(on_wait=[w], on_update=[]),
                        bass_nofuse=True,
                        engine=ins.engine,
                    )
                    out.append(nop)
                    changed = True
                ins.sync_info = mybir.SyncInfo(
                    on_wait=keep, on_update=list(si.on_update)
                )
            out.append(ins)
        if changed:
            bb.instructions = out
    return nc


_PRUNABLE_UPDATERS = (
    "InstMatmult",
    "InstActivation",
    "InstReciprocal",
    "InstTensorScalarPtr",
    "InstTensorScalar",
    "InstMemset",
)


def _prune_sem_updates(nc):
    f = nc.m.functions[0]
    all_insts = [ins for bb in f.blocks for ins in bb.instructions]
    referenced = {}
    for ins in all_insts:
        si = ins.sync_info
        if si:
            for w in si.on_wait:
                referenced.setdefault(w.id, set()).add(w.wait_value)
    from collections import defaultdict

    upd = defaultdict(list)
    untouchable = set()
    for ins in all_insts:
        si = ins.sync_info
        if not si:
            continue
        for u in si.on_update:
            upd[u.id].append(ins)
            if type(ins).__name__ not in _PRUNABLE_UPDATERS or u.update_value != 1:
                untouchable.add(u.id)
    for sem_id, lst in upd.items():
        if sem_id in untouchable:
            continue
        n = len(lst)
        refs = referenced.get(sem_id, set())
        kept = sorted(v for v in refs if 1 <= v <= n)
        if not kept or kept[-1] != n:
            kept.append(n)
        kept_set = set(kept)
        rank = {v: i + 1 for i, v in enumerate(kept)}
        for tick, ins in enumerate(lst, start=1):
            if tick in kept_set:
                continue
            si = ins.sync_info
            ins.sync_info = mybir.SyncInfo(
                on_wait=list(si.on_wait),
                on_update=[u for u in si.on_update if u.id != sem_id],
            )
        for ins in all_insts:
            si = ins.sync_info
            if not si or not any(w.id == sem_id for w in si.on_wait):
                continue
            new_waits = []
            for w in si.on_wait:
                if w.id == sem_id:
                    w = mybir.SyncWait(
                        sync_type=w.sync_type,
                        id=w.id,
                        ant_name=w.ant_name,
                        wait_mode=w.wait_mode,
                        wait_value=rank[w.wait_value],
                        wait_reg=w.wait_reg,
                    )
                new_waits.append(w)
            ins.sync_info = mybir.SyncInfo(
                on_wait=new_waits, on_update=list(si.on_update)
            )
    return nc


def _build_bass():
    nc = bass.Bass()
    qT_d = nc.declare_dram_parameter("qT", [HP, 128, S], BF16, isOutput=False)
    kT_d = nc.declare_dram_parameter("kT", [HP, 128, S], BF16, isOutput=False)
    vp_d = nc.declare_dram_parameter("vp", [G, 128, NT, D + 1], BF16, isOutput=False)
    if L4:
        out_d = nc.declare_dram_parameter(
            "out", [G, D + 1, S // 2], F32, isOutput=True
        )
    else:
        out_d = nc.declare_dram_parameter("out", [G, D + 1, S], F32, isOutput=True)

    with tile.TileContext(nc) as tc:
        with (
            tc.tile_pool(name="const", bufs=1) as c_pool,
            tc.tile_pool(name="qk", bufs=2) as qk_pool,
            tc.tile_pool(name="vpool", bufs=4 if (L6 or L7) else 2) as v_pool,
            tc.tile_pool(name="opool", bufs=4 if (L6 or L7) else 2) as o_pool,
            tc.tile_pool(
                name="ppool", bufs=max(8 if (L6 or L7) else 4, AVLAG + 3)
            ) as p_pool,
            tc.tile_pool(name="stps", bufs=2, space="PSUM") as st_pool,
            tc.tile_pool(name="otps", bufs=2, space="PSUM") as ot_pool,
        ):
            bias0 = c_pool.tile([128, 1], F32, name="bias0")
            nc.vector.memset(bias0, 0.0)
            scratch0 = c_pool.tile([128, 1], F32, name="scratch0")
            nc.scalar.activation(
                scratch0, bias0, mybir.ActivationFunctionType.Exp, bias=bias0
            )

            if L7:
                # Interleave the two pairs of each head-pair quad-by-quad:
                # consecutive ST matmuls alternate PE row halves, so the
                # next strip's LDWEIGHTS overlaps the in-flight matmul.
                units = [
                    (2 * hp + h, q)
                    for hp in range(HP)
                    for q in range(QUADS)
                    for h in range(2)
                ]
            else:
                units = [(g, q) for g in range(G) for q in range(QUADS)]
            uidx = {gq: j for j, gq in enumerate(units)}
            qkv = {}
            hp_qk = {}
            p_t = {}

            def load_hp(hp):
                qT_sb = qk_pool.tile([128, S], BF16, tag="qT", name=f"qT{hp}")
                kT_sb = qk_pool.tile([128, S], BF16, tag="kT", name=f"kT{hp}")
                C0 = 1024
                if hp == 0:
                    # leading columns gate the very first matmuls; split
                    # across queues (one dma_start rides one ~17GB/s queue)
                    for a, b in ((0, 128), (128, 256), (256, 384), (384, 512)):
                        nc.sync.dma_start(out=qT_sb[:, a:b], in_=qT_d[hp][:, a:b])
                        nc.sync.dma_start(out=kT_sb[:, a:b], in_=kT_d[hp][:, a:b])
                    nc.sync.dma_start(out=qT_sb[:, 512:C0], in_=qT_d[hp][:, 512:C0])
                    nc.sync.dma_start(out=kT_sb[:, 512:C0], in_=kT_d[hp][:, 512:C0])
                else:
                    nc.sync.dma_start(out=qT_sb[:, 0:C0], in_=qT_d[hp][:, 0:C0])
                    nc.sync.dma_start(out=kT_sb[:, 0:C0], in_=kT_d[hp][:, 0:C0])
                nc.sync.dma_start(out=qT_sb[:, C0 : S // 2], in_=qT_d[hp][:, C0 : S // 2])
                nc.sync.dma_start(out=kT_sb[:, C0 : S // 2], in_=kT_d[hp][:, C0 : S // 2])
                nc.sync.dma_start(out=qT_sb[:, S // 2 :], in_=qT_d[hp][:, S // 2 :])
                nc.sync.dma_start(out=kT_sb[:, S // 2 :], in_=kT_d[hp][:, S // 2 :])
                hp_qk[hp] = (qT_sb, kT_sb)

            def load_pair(g):
                vp_sb = v_pool.tile([128, NT, D + 1], BF16, tag="vp", name=f"vp{g}")
                nc.sync.dma_start(
                    out=vp_sb[:, 0 : NT // 2, :], in_=vp_d[g][:, 0 : NT // 2, :]
                )
                nc.sync.dma_start(
                    out=vp_sb[:, NT // 2 :, :], in_=vp_d[g][:, NT // 2 :, :]
                )
                out_sb = o_pool.tile(
                    [D + 1, S], BF16 if L4 else F32, tag="osb", name=f"o{g}"
                )
                qkv[g] = (vp_sb, out_sb)

            def emit_st7(j):
                g, quad = units[j]
                hp = g // 2
                rh = 64 * (g % 2)
                if j == 0:
                    load_hp(0)
                    load_pair(0)
                    load_pair(1)
                if j == 8:
                    # prefetch the second head-pair mid-way through the
                    # first so its 3MB doesn't stall the handover
                    load_hp(1)
                    load_pair(2)
                    load_pair(3)
                qT_sb, kT_sb = hp_qk[hp]
                vp_sb, out_sb = qkv[g]
                st = st_pool.tile([128, 1536], F32, tag="st", name=f"st{j}")
                p_sb = p_pool.tile([128, 1536], BF16, tag="p", name=f"p{j}")
                p_t[j] = p_sb
                for s in range(4):
                    c = quad * 4 + s
                    base = _PERM[s] * 384
                    t_lo = max(0, c - 1)
                    t_hi = min(NT, c + 2)
                    p0 = base + (t_lo - (c - 1)) * 128
                    bnd = base + (t_hi - (c - 1)) * 128
                    while p0 < bnd:
                        p1 = min(bnd, (p0 // 512 + 1) * 512)
                        q0 = (c - 1) * 128 + (p0 - base)
                        nc.tensor.matmul(
                            st[:, p0:p1],
                            lhsT=kT_sb[rh : rh + 64, c * 128 : (c + 1) * 128],
                            rhs=qT_sb[rh : rh + 64, q0 : q0 + (p1 - p0)],
                            start=True,
                            stop=True,
                        )
                        p0 = p1
                lo = 128 if quad == 0 else 0
                hi = 1536 - 128 if quad == QUADS - 1 else 1536
                if L3 and EXP_ENGINE[j] == 'V':
                    nc.vector.tensor_scalar(
                        p_sb[:, lo:hi].bitcast(I16),
                        st[:, lo:hi],
                        FE_A,
                        FE_B,
                        mybir.AluOpType.mult,
                        mybir.AluOpType.add,
                    )
                else:
                    nc.scalar.activation(
                        p_sb[:, lo:hi],
                        st[:, lo:hi],
                        mybir.ActivationFunctionType.Exp,
                        bias=bias0,
                        scale=1.0 / np.sqrt(D).item(),
                    )
                # strip 0's low corner gates the PREVIOUS quad's AV tail
                # (cross-quad chunk read); emit the low corners first.
                for s in range(4):
                    c = quad * 4 + s
                    base = _PERM[s] * 384
                    if c >= 1:
                        nc.gpsimd.memset(p_sb[64:128, base : base + 64], 0.0)
                for s in range(4):
                    c = quad * 4 + s
                    base = _PERM[s] * 384
                    if c <= NT - 2:
                        nc.gpsimd.memset(p_sb[0:64, base + 320 : base + 384], 0.0)

            def emit_st(j):
                g, quad = units[j]
                hp = g // 2
                rh = 64 * (g % 2)
                if quad == 0 and g % 2 == 0:
                    qT_sb = qk_pool.tile([128, S], BF16, tag="qT", name=f"qT{hp}")
                    kT_sb = qk_pool.tile([128, S], BF16, tag="kT", name=f"kT{hp}")
                    C0 = 1024
                    if hp == 0:
                        # the very first ST waits on these columns; split
                        # them across queues (a single dma_start rides one
                        # ~17GB/s queue) so compute starts ~4us earlier
                        for a, b in ((0, 128), (128, 256), (256, 384), (384, 512)):
                            nc.sync.dma_start(out=qT_sb[:, a:b], in_=qT_d[hp][:, a:b])
                            nc.sync.dma_start(out=kT_sb[:, a:b], in_=kT_d[hp][:, a:b])
                        nc.sync.dma_start(out=qT_sb[:, 512:C0], in_=qT_d[hp][:, 512:C0])
                        nc.sync.dma_start(out=kT_sb[:, 512:C0], in_=kT_d[hp][:, 512:C0])
                    else:
                        nc.sync.dma_start(out=qT_sb[:, 0:C0], in_=qT_d[hp][:, 0:C0])
                        nc.sync.dma_start(out=kT_sb[:, 0:C0], in_=kT_d[hp][:, 0:C0])
                    nc.sync.dma_start(
                        out=qT_sb[:, C0 : S // 2], in_=qT_d[hp][:, C0 : S // 2]
                    )
                    nc.sync.dma_start(
                        out=kT_sb[:, C0 : S // 2], in_=kT_d[hp][:, C0 : S // 2]
                    )
                    nc.sync.dma_start(out=qT_sb[:, S // 2 :], in_=qT_d[hp][:, S // 2 :])
                    nc.sync.dma_start(out=kT_sb[:, S // 2 :], in_=kT_d[hp][:, S // 2 :])
                    hp_qk[hp] = (qT_sb, kT_sb)
                if quad == 0:
                    vp_sb = v_pool.tile([128, NT, D + 1], BF16, tag="vp", name=f"vp{g}")
                    nc.sync.dma_start(
                        out=vp_sb[:, 0 : NT // 2, :], in_=vp_d[g][:, 0 : NT // 2, :]
                    )
                    nc.sync.dma_start(
                        out=vp_sb[:, NT // 2 :, :], in_=vp_d[g][:, NT // 2 :, :]
                    )
                    out_sb = o_pool.tile(
                        [D + 1, S], BF16 if L4 else F32, tag="osb", name=f"o{g}"
                    )
                    qkv[g] = (vp_sb, out_sb)
                qT_sb, kT_sb = hp_qk[hp]
                vp_sb, out_sb = qkv[g]
                st = st_pool.tile([128, 1536], F32, tag="st", name=f"st{j}")
                p_sb = p_pool.tile([128, 1536], BF16, tag="p", name=f"p{j}")
                p_t[j] = p_sb
                for s in range(4):
                    c = quad * 4 + s
                    base = _PERM[s] * 384
                    t_lo = max(0, c - 1)
                    t_hi = min(NT, c + 2)
                    p0 = base + (t_lo - (c - 1)) * 128
                    bnd = base + (t_hi - (c - 1)) * 128
                    pieces = []
                    while p0 < bnd:
                        p1 = min(bnd, (p0 // 512 + 1) * 512)
                        pieces.append((p0, p1))
                        p0 = p1
                    for p0, p1 in pieces:
                        q0 = (c - 1) * 128 + (p0 - base)
                        nc.tensor.matmul(
                            st[:, p0:p1],
                            lhsT=kT_sb[rh : rh + 64, c * 128 : (c + 1) * 128],
                            rhs=qT_sb[rh : rh + 64, q0 : q0 + (p1 - p0)],
                            start=True,
                            stop=True,
                        )
                lo = 128 if quad == 0 else 0
                hi = 1536 - 128 if quad == QUADS - 1 else 1536
                if L3 and EXP_ENGINE[j] == 'V':
                    nc.vector.tensor_scalar(
                        p_sb[:, lo:hi].bitcast(I16),
                        st[:, lo:hi],
                        FE_A,
                        FE_B,
                        mybir.AluOpType.mult,
                        mybir.AluOpType.add,
                    )
                else:
                    nc.scalar.activation(
                        p_sb[:, lo:hi],
                        st[:, lo:hi],
                        mybir.ActivationFunctionType.Exp,
                        bias=bias0,
                        scale=1.0 / np.sqrt(D).item(),
                    )
                for s in range(4):
                    c = quad * 4 + s
                    base = _PERM[s] * 384
                    if c >= 1:
                        nc.gpsimd.memset(p_sb[64:128, base : base + 64], 0.0)
                for s in range(4):
                    c = quad * 4 + s
                    base = _PERM[s] * 384
                    if c <= NT - 2:
                        nc.gpsimd.memset(p_sb[0:64, base + 320 : base + 384], 0.0)

            def emit_av(j):
                g, quad = units[j]
                vp_sb, out_sb = qkv[g]
                ot = ot_pool.tile([D + 1, 512], F32, tag="ot", name=f"ot{j}")
                t0 = quad * 4
                mms = []
                for c in range(max(0, t0 - 1), min(NT, t0 + 5)):
                    t_lo = max(t0, c - 1, 0)
                    t_hi = min(t0 + 4, c + 2, NT)
                    if t_lo >= t_hi:
                        continue
                    pq = p_t[uidx[(g, c // 4)]]
                    r0 = _PERM[c % 4] * 384 + (t_lo - (c - 1)) * 128
                    r1 = _PERM[c % 4] * 384 + (t_hi - (c - 1)) * 128
                    mms.append(
                        (
                            ot[:, (t_lo - t0) * 128 : (t_hi - t0) * 128],
                            vp_sb[:, c, :],
                            pq[:, r0:r1],
                        )
                    )
                for i, (o, w, r) in enumerate(mms):
                    nc.tensor.matmul(
                        o,
                        lhsT=w,
                        rhs=r,
                        start=(i == 0),
                        stop=(i == len(mms) - 1),
                        skip_group_check=True,
                    )
                if L2 and EVICT_ENGINE[j] == 'S':
                    nc.scalar.copy(out_sb[:, quad * 512 : (quad + 1) * 512], ot[:, :])
                else:
                    nc.vector.tensor_copy(
                        out_sb[:, quad * 512 : (quad + 1) * 512], ot[:, :]
                    )
                p_t.pop(j - (4 if L7 else 1), None)
                if quad % 2 == 1:
                    # the final 2-quad chunk is the kernel's drain tail;
                    # split it so the last transfer parallelizes
                    if quad == QUADS - 1:
                        spans = [
                            ((quad - 1) * 512 + 256 * t, (quad - 1) * 512 + 256 * (t + 1))
                            for t in range(4)
                        ]
                    else:
                        spans = [((quad - 1) * 512, (quad + 1) * 512)]
                    for a, b in spans:
                        if L4:
                            nc.sync.dma_start(
                                out=out_d[g][:, a // 2 : b // 2],
                                in_=out_sb[:, a:b].bitcast(F32),
                            )
                        else:
                            nc.sync.dma_start(
                                out=out_d[g][:, a:b], in_=out_sb[:, a:b]
                            )

            # L6: units of (head-pair, 2 chunks) with the two heads' strips
            # interleaved so consecutive ST matmuls alternate PE row halves
            # (next strip's LDWEIGHTS overlaps the in-flight matmul).
            # st/p layout: col = h*768 + s*384 + w, chunk c = 2u+s.
            def emit_unit6(g_idx):
                hp, u = g_idx % 2, g_idx // 2
                if u == 0:
                    qT_sb = qk_pool.tile([128, S], BF16, tag="qT", name=f"qT{hp}")
                    kT_sb = qk_pool.tile([128, S], BF16, tag="kT", name=f"kT{hp}")
                    C0 = 1024
                    nc.sync.dma_start(out=qT_sb[:, 0:C0], in_=qT_d[hp][:, 0:C0])
                    nc.sync.dma_start(out=kT_sb[:, 0:C0], in_=kT_d[hp][:, 0:C0])
                    nc.sync.dma_start(out=qT_sb[:, C0 : S // 2], in_=qT_d[hp][:, C0 : S // 2])
                    nc.sync.dma_start(out=kT_sb[:, C0 : S // 2], in_=kT_d[hp][:, C0 : S // 2])
                    nc.sync.dma_start(out=qT_sb[:, S // 2 :], in_=qT_d[hp][:, S // 2 :])
                    nc.sync.dma_start(out=kT_sb[:, S // 2 :], in_=kT_d[hp][:, S // 2 :])
                    hp_qk[hp] = (qT_sb, kT_sb)
                    for h in range(2):
                        g = 2 * hp + h
                        vp_sb = v_pool.tile(
                            [128, NT, D + 1], BF16, tag="vp", name=f"vp{g}"
                        )
                        nc.sync.dma_start(
                            out=vp_sb[:, 0 : NT // 2, :], in_=vp_d[g][:, 0 : NT // 2, :]
                        )
                        nc.sync.dma_start(
                            out=vp_sb[:, NT // 2 :, :], in_=vp_d[g][:, NT // 2 :, :]
                        )
                        out_sb = o_pool.tile(
                            [D + 1, S], BF16 if L4 else F32, tag="osb", name=f"o{g}"
                        )
                        qkv[g] = (vp_sb, out_sb)
                qT_sb, kT_sb = hp_qk[hp]
                st = st_pool.tile([128, 1536], F32, tag="st", name=f"st{g_idx}")
                p_sb = p_pool.tile([128, 1536], BF16, tag="p", name=f"p{g_idx}")
                p_t[g_idx] = p_sb
                for s in range(2):
                    c = 2 * u + s
                    for h in range(2):
                        base = h * 768 + s * 384
                        t_lo = max(0, c - 1)
                        t_hi = min(NT, c + 2)
                        p0 = base + (t_lo - (c - 1)) * 128
                        bnd = base + (t_hi - (c - 1)) * 128
                        while p0 < bnd:
                            p1 = min(bnd, (p0 // 512 + 1) * 512)
                            q0 = (c - 1) * 128 + (p0 - base)
                            nc.tensor.matmul(
                                st[:, p0:p1],
                                lhsT=kT_sb[64 * h : 64 * h + 64, c * 128 : (c + 1) * 128],
                                rhs=qT_sb[64 * h : 64 * h + 64, q0 : q0 + (p1 - p0)],
                                start=True,
                                stop=True,
                            )
                            p0 = p1
                # exp; first/last units trim the never-written hole columns
                # (uninitialized PSUM reads can fault the device).
                if u == 0:
                    ranges = [(128, 768), (896, 1536)]
                elif u == NT // 2 - 1:
                    ranges = [(0, 640), (768, 1408)]
                else:
                    ranges = [(0, 1536)]
                for lo, hi in ranges:
                    if L3 and EXP_ENGINE[g_idx] == 'V':
                        nc.vector.tensor_scalar(
                            p_sb[:, lo:hi].bitcast(I16),
                            st[:, lo:hi],
                            FE_A,
                            FE_B,
                            mybir.AluOpType.mult,
                            mybir.AluOpType.add,
                        )
                    else:
                        nc.scalar.activation(
                            p_sb[:, lo:hi],
                            st[:, lo:hi],
                            mybir.ActivationFunctionType.Exp,
                            bias=bias0,
                            scale=1.0 / np.sqrt(D).item(),
                        )
                for s in range(2):
                    c = 2 * u + s
                    for h in range(2):
                        base = h * 768 + s * 384
                        if c <= NT - 2:
                            nc.gpsimd.memset(p_sb[0:64, base + 320 : base + 384], 0.0)
                        if c >= 1:
                            nc.gpsimd.memset(p_sb[64:128, base : base + 64], 0.0)

            av6_count = [0]

            def emit_av6(hp, h, jq):
                g = 2 * hp + h
                vp_sb, out_sb = qkv[g]
                ot = ot_pool.tile([D + 1, 512], F32, tag="ot", name=f"ot{g}_{jq}")
                t0 = 4 * jq
                mms = []
                for c in range(max(0, t0 - 1), min(NT, t0 + 5)):
                    t_lo = max(t0, c - 1, 0)
                    t_hi = min(t0 + 4, c + 2, NT)
                    if t_lo >= t_hi:
                        continue
                    pq = p_t[2 * (c // 2) + hp]
                    base = h * 768 + (c % 2) * 384
                    r0 = base + (t_lo - (c - 1)) * 128
                    r1 = base + (t_hi - (c - 1)) * 128
                    mms.append(
                        (
                            ot[:, (t_lo - t0) * 128 : (t_hi - t0) * 128],
                            vp_sb[:, c, :],
                            pq[:, r0:r1],
                        )
                    )
                for i, (o, w, r) in enumerate(mms):
                    nc.tensor.matmul(
                        o,
                        lhsT=w,
                        rhs=r,
                        start=(i == 0),
                        stop=(i == len(mms) - 1),
                        skip_group_check=True,
                    )
                ev = av6_count[0]
                av6_count[0] += 1
                if L2 and EVICT_ENGINE[ev % 32] == 'S':
                    nc.scalar.copy(out_sb[:, jq * 512 : (jq + 1) * 512], ot[:, :])
                else:
                    nc.vector.tensor_copy(
                        out_sb[:, jq * 512 : (jq + 1) * 512], ot[:, :]
                    )
                if jq % 2 == 1:
                    sl = slice((jq - 1) * 512, (jq + 1) * 512)
                    if L4:
                        f_sl = slice((jq - 1) * 256, (jq + 1) * 256)
                        nc.sync.dma_start(
                            out=out_d[g][:, f_sl], in_=out_sb[:, sl].bitcast(F32)
                        )
                    else:
                        nc.sync.dma_start(out=out_d[g][:, sl], in_=out_sb[:, sl])

            if L6:
                av_fifo = []
                for g_idx in range(2 * (NT // 2)):
                    emit_unit6(g_idx)
                    hp, u = g_idx % 2, g_idx // 2
                    if u >= 2 and u % 2 == 0:
                        jq = u // 2 - 1
                        av_fifo.append((hp, 0, jq))
                        av_fifo.append((hp, 1, jq))
                    if av_fifo:
                        emit_av6(*av_fifo.pop(0))
                    p_t.pop(g_idx - 8, None)
                av_fifo.append((0, 0, QUADS - 1))
                av_fifo.append((0, 1, QUADS - 1))
                av_fifo.append((1, 0, QUADS - 1))
                av_fifo.append((1, 1, QUADS - 1))
                for av in av_fifo:
                    emit_av6(*av)
            else:
                st_fn = emit_st7 if L7 else emit_st
                for j in range(len(units)):
                    st_fn(j)
                    if j >= AVLAG:
                        emit_av(j - AVLAG)
                for jj in range(len(units) - AVLAG, len(units)):
                    emit_av(jj)
    _split_excess_waits(nc)
    return _prune_sem_updates(nc)


def _prep_inputs(q, k, v):
    bf16 = ml_dtypes.bfloat16
    qb = np.ascontiguousarray(np.asarray(q).transpose(0, 2, 1, 3).reshape(GH, S, D))
    kb = np.ascontiguousarray(np.asarray(k).transpose(0, 2, 1, 3).reshape(GH, S, D))
    vb = np.ascontiguousarray(np.asarray(v).transpose(0, 2, 1, 3).reshape(GH, S, D))

    qT = np.ascontiguousarray(qb.transpose(0, 2, 1)).astype(bf16).reshape(GH // 2, 128, S)
    kT = np.ascontiguousarray(kb.transpose(0, 2, 1)).astype(bf16).reshape(GH // 2, 128, S)
    v4 = vb.reshape(GH, NT, 128, D).transpose(0, 2, 1, 3)
    vp = np.empty((GH, 128, NT, D + 1), dtype=bf16)
    vp[..., :D] = v4.astype(bf16)
    vp[..., D] = np.array(1.0, dtype=bf16)

    in_maps = []
    for cc in range(N_CORES):
        in_maps.append(
            {
                "qT": np.ascontiguousarray(qT[cc * HP : (cc + 1) * HP]),
                "kT": np.ascontiguousarray(kT[cc * HP : (cc + 1) * HP]),
                "vp": np.ascontiguousarray(vp[cc * G : (cc + 1) * G]),
            }
        )
    return in_maps


def _assemble_output(results):
    o = np.concatenate([np.asarray(r["out"]) for r in results], axis=0)
    if L4:
        o = o.view(ml_dtypes.bfloat16).astype(np.float32)  # [GH, D+1, S]
    o = o[:, :D, :] / o[:, D : D + 1, :]
    o = o.transpose(0, 2, 1)
    o = o.reshape(B, H, S, D).transpose(0, 2, 1, 3)
    return np.ascontiguousarray(o.astype(np.float32))


def _run(q, k, v, trace=False, tmpdir=None):
    global _nc_cache
    if _nc_cache is None:
        _nc_cache = _build_bass()
    in_maps = _prep_inputs(q, k, v)
    res = run_bass_kernel_spmd(
        _nc_cache, in_maps, core_ids=list(range(N_CORES)), trace=trace, tmpdir=tmpdir
    )
    return _assemble_output(res.results), res.exec_time_ns


def kernel(q, k, v):
    out, _ = _run(q, k, v)
    return out
